# revision 1
# baseline (speedup 1.0000x reference)
"""Multi-head scaled-dot-product attention (ABSA-style, per-head projections)
on 8 Trainium2 NeuronCores.

Reference computation (per head h, batch b):
    kx = k @ w_kx[h]                    # (512, 96)
    qx = q @ w_qx[h]                    # (512, 96)
    s  = qx @ kx.T / sqrt(96)           # (512, 512)
    a  = softmax(s, axis=-1)
    o  = a @ kx                         # (512, 96)
    out[b, :, h*96:(h+1)*96] = o

Distribution: data-parallel over batch. 32 batches are split 4-per-core over
8 cores; every core holds the full (tiny) weights and computes all 8 heads
for its 4 batches. No collectives needed — the host concatenates the
per-core outputs.

Per-core dataflow (matmuls in bf16, accumulation + softmax math in f32):
  - SWDGE cast-DMAs load k, q, and the weights f32 -> bf16 straight into
    SBUF (contiguous descriptors, no DRAM bounce).  kT/qT (embed on
    partitions) are built with PE transposes (~52 ns per 128x128 block) —
    DMA xbar transposes were measured ~1.26 us each and globally fence the
    DMA queues against plain copies, so they are avoided entirely.
  - Projections run as 6 accumulating matmuls per (h,b) with the natural
    (embed, hidden) weight layout as the stationary operand, producing
    kx^T/qx^T (hidden, seq) directly — no on-chip weight transposes.
  - Scores are computed transposed, s^T (k, q), so the softmax reduction
    axis lands on PSUM partitions and is folded into the second matmul:
    kx is augmented with a ones column (via a 97-row PE transpose), so the
    attention matmul produces both sum_k exp*kx and sum_k exp (the softmax
    denominator) in one accumulation group.  exp() runs unshifted — scores
    are O(1) by construction so there is no overflow risk — which removes
    the need for any cross-partition max reduction.
  - Normalisation (multiply by the reciprocal of column 96) happens on the
    PSUM->SBUF eviction path into a per-batch staging tile; one contiguous
    DMA per 128 query rows writes all 8 heads at once.

Per batch, the loop is phase-split: all 8 q-projections first (their
inputs land earliest, filling the PE while k-side casts are in flight),
then the full k-side chain (projection -> PE transpose -> eviction), then
the attention phase (scores -> exp -> weighted sum) per head.

Measured on 8 axon-tunneled TRN2 NeuronCores: ~200-206 us NEFF exec time
(~87%% TensorEngine occupancy), L2 relative error 3.9e-3 vs the fp32
reference.
"""

import math
from functools import lru_cache

import numpy as np

import concourse.bass as bass
import concourse.tile as tile
from concourse import mybir
from concourse.bass_utils import run_bass_kernel_spmd
from concourse.masks import make_identity

# ---------------------------------------------------------------------------
# Workaround for walrus "Too many sync wait commands": some instruction
# encodings accept only a single sync-wait, but Tile can attach several
# (e.g. the tail drain, or transpose DMAs gated on both their producer and
# the xbar-mode serialisation).  Hoist every wait beyond the first onto a
# same-engine no-op inserted right before the instruction — program order on
# the engine makes that equivalent.
# ---------------------------------------------------------------------------

import bass_rust as _bass_rust


def _split_excess_waits(nc, max_waits=1):
    n = 0
    for f in nc.m.functions:
        for bb in f.blocks:
            il = bb.instructions
            i = 0
            while i < len(il):
                ins = il[i]
                si = ins.sync_info
                waits = list(si.on_wait or []) if si is not None else []
                if len(waits) > max_waits:
                    si.on_wait = waits[:max_waits]
                    for w in waits[max_waits:]:
                        nop = mybir.InstNoOp(name=f"waitnop-{n}", ins=[],
                                             outs=[])
                        n += 1
                        nop.engine = ins.engine
                        nop.sync_info = _bass_rust.SyncInfo(
                            on_wait=[w], on_update=[])
                        il.insert(i, nop)
                        i += 1
                i += 1

# ---------------------------------------------------------------------------
# Problem constants (full problem; hardcoded per the harness contract)
# ---------------------------------------------------------------------------
EMBED = 768
HID = 96
N_HEAD = 8
BATCH = 32
SEQ = 512
N_CORES = 8
B = BATCH // N_CORES  # batches per core
EC = EMBED // 128  # embed chunks of 128
KC = SEQ // 128  # key chunks of 128
QC = SEQ // 128  # query chunks of 128
SCALE = 1.0 / math.sqrt(HID)

F32 = mybir.dt.float32
BF16 = mybir.dt.bfloat16


def build_bass():
    nc = bass.Bass("TRN2", target_bir_lowering=False, debug=False,
                   num_devices=N_CORES)

    k_in = nc.declare_dram_parameter("k", [B, SEQ, EMBED], F32, isOutput=False)
    q_in = nc.declare_dram_parameter("q", [B, SEQ, EMBED], F32, isOutput=False)
    wk_in = nc.declare_dram_parameter("w_kx", [N_HEAD, EMBED, HID], F32,
                                      isOutput=False)
    wq_in = nc.declare_dram_parameter("w_qx", [N_HEAD, EMBED, HID], F32,
                                      isOutput=False)
    out_d = nc.declare_dram_parameter("out", [B, SEQ, EMBED], F32,
                                      isOutput=True)

    with nc.allow_low_precision("bf16 compute, f32 accumulate"), \
            tile.TileContext(nc) as tc:
        with tc.tile_pool(name="singles", bufs=1) as singles, \
                tc.tile_pool(name="nat", bufs=4) as nat_pool, \
                tc.tile_pool(name="kqt", bufs=1) as kqt_pool, \
                tc.tile_pool(name="wsb", bufs=1) as w_pool, \
                tc.tile_pool(name="stage", bufs=1) as stage_pool, \
                tc.tile_pool(name="qx", bufs=3) as qx_pool, \
                tc.tile_pool(name="exp", bufs=8) as exp_pool, \
                tc.tile_pool(name="kxo", bufs=10) as kxo_pool, \
                tc.tile_pool(name="recip", bufs=8) as recip_pool, \
                tc.tile_pool(name="ps_proj", bufs=2, space="PSUM") as ps_proj, \
                tc.tile_pool(name="ps_score", bufs=2, space="PSUM") as ps_score, \
                tc.tile_pool(name="ps_trans", bufs=2, space="PSUM") as ps_trans, \
                tc.tile_pool(name="ps_out", bufs=2, space="PSUM") as ps_out:

            # --- one-time setup -------------------------------------------
            identity = singles.tile([128, 128], BF16, tag="identity")
            make_identity(nc, identity[:])
            # PE warm-up: dummy transposes fill the otherwise-idle startup
            # window (input casts in flight), flipping the HAM clock gate to
            # 2.4 GHz before the first real matmuls.
            warm_ps = ps_score.tile([128, 256], BF16, tag="score",
                                    name="warm_ps")
            for _ in range(32):
                nc.tensor.transpose(warm_ps[:, 0:128], identity[:],
                                    identity[:])

            # kx^T staging tiles with a persistent ones-row (row 96).  Two
            # tiles, alternated per (b,h) iteration for pipelining.
            # kx^T staging tiles (two, alternated per iteration).  The
            # softmax-denominator ones column is memset into kxo every
            # iteration instead of kept as persistent state: init-once
            # SBUF state proved racy on the first execution of a NEFF.
            kx97 = [[singles.tile([HID, SEQ], BF16, tag=f"kx97_{i}_{h}",
                                  name=f"kx97_{i}_{h}")
                     for h in range(N_HEAD)] for i in range(2)]

            # --- input pipeline -------------------------------------------
            # All loads are SWDGE cast-DMAs (f32 -> bf16) with contiguous
            # descriptors.  Emission order puts w_qx and batch 0 first so
            # the PE can start at ~20 us.  kT/qT are built by PE transposes
            # from the natural-layout SBUF tiles.  Weight layout
            # (128, 48, 96): block t = 6*h + ec holds head h, embed chunk
            # ec.
            w_sb = [w_pool.tile([128, EC * N_HEAD, HID], BF16,
                                tag=f"w{t}", name=f"wbf{t}")
                    for t in range(2)]
            kT = {}
            qT = {}

            def load_w_one(t, half):
                # SWDGE cast-load straight to bf16, two halves so the first
                # heads' weights land early.
                w_in = (wk_in, wq_in)[t]
                src = w_in.rearrange("h e d -> (h e) d").rearrange(
                    "(t p) d -> p t d", p=128)
                HB = EC * N_HEAD // 2
                sl = slice(half * HB, (half + 1) * HB)
                nc.gpsimd.dma_start(out=w_sb[t][:, sl, :],
                                    in_=src[:, sl, :])

            def cast_batch_tensor(b, t):
                src_d = (k_in, q_in)[t]
                nat = nat_pool.tile([128, KC, EMBED], BF16,
                                    tag=f"nat{t}", name=f"nat{t}_{b}")
                nc.gpsimd.dma_start(
                    out=nat[:],
                    in_=src_d[b].rearrange("(kc p) e -> p kc e", p=128))
                return nat

            def transpose_batch_tensor(b, t, nat):
                # PE transposes (52 ns per 128x128 block) build the
                # embed-on-partitions kT/qT tiles; no DMA transposes at all.
                dst = qT if t else kT
                for ec in range(EC):
                    tp = ps_trans.tile([128, KC, 128], BF16,
                                       tag="tr", name="kt_tr")
                    for kc in range(KC):
                        nc.tensor.transpose(
                            tp[:, kc, :],
                            nat[:, kc, ec * 128:(ec + 1) * 128],
                            identity[:])
                    tt = kqt_pool.tile(
                        [128, SEQ], BF16,
                        tag=f"T{t}_{b}_{ec}", name=f"T{t}_{b}_{ec}")
                    if ec % 2:
                        nc.vector.tensor_copy(tt[:], tp[:])
                    else:
                        nc.scalar.copy(tt[:], tp[:])
                    dst[b, ec] = tt

            def load_batch(b):
                for t in (1, 0):
                    nat = cast_batch_tensor(b, t)
                    transpose_batch_tensor(b, t, nat)

            # SWDGE queue order tuned for startup: q0 cast, w_qx halves,
            # k0 cast, w_kx halves, then the remaining batches.
            nat_q0 = cast_batch_tensor(0, 1)
            load_w_one(1, 0)
            nat_k0 = cast_batch_tensor(0, 0)
            load_w_one(1, 1)
            transpose_batch_tensor(0, 1, nat_q0)
            load_w_one(0, 0)
            load_w_one(0, 1)
            transpose_batch_tensor(0, 0, nat_k0)
            for b in range(1, B):
                load_batch(b)

            # Output staging: (128, EMBED) f32 per (batch parity, q chunk).
            stage = [[stage_pool.tile([128, EMBED], F32, tag=f"st{p}_{qc}", name=f"st{p}_{qc}")
                      for qc in range(QC)] for p in range(2)]

            # --- main loop ------------------------------------------------
            # per-(parity, head) qx^T tiles so the whole q-projection
            # phase can run before the k-side inputs arrive
            qxs = [[singles.tile([HID, SEQ], BF16, tag=f"qxs_{i}_{h}",
                                 name=f"qxs_{i}_{h}")
                    for h in range(N_HEAD)] for i in range(2)]

            it = 0
            for b in range(B):
                par = b % 2
                st = stage[par]
                # q-projection phase: needs only qT(b) + w_qx, which land
                # first — fills the PE while the k-side casts are in flight
                for h in range(N_HEAD):
                    qx_ps = ps_proj.tile([HID, SEQ], F32, tag="proj", name="proj_ps")
                    for ec in range(EC):
                        nc.tensor.matmul(qx_ps[:], w_sb[1][:, h * EC + ec, :],
                                         qT[b, ec][:],
                                         start=(ec == 0), stop=(ec == EC - 1))
                    nc.scalar.copy(qxs[par][h][:], qx_ps[:])

                # k-projection phase: kx^T, then kx-natural (+ones) tiles
                kxos = []
                for h in range(N_HEAD):
                    kx_ps = ps_proj.tile([HID, SEQ], F32, tag="proj", name="proj_ps")
                    for ec in range(EC):
                        nc.tensor.matmul(kx_ps[:], w_sb[0][:, h * EC + ec, :],
                                         kT[b, ec][:],
                                         start=(ec == 0), stop=(ec == EC - 1))
                    kx97_t = kx97[par][h]
                    nc.vector.tensor_copy(kx97_t[:], kx_ps[:])

                    tr_ps = ps_trans.tile([128, KC, HID + 2], BF16, tag="tr", name="tr_ps")
                    for kc in range(KC):
                        nc.tensor.transpose(
                            tr_ps[:, kc, 0:HID],
                            kx97_t[:, kc * 128:(kc + 1) * 128],
                            identity[0:HID, 0:HID])
                    kxo = kxo_pool.tile([128, KC, HID + 2], BF16, tag="kxo", name="kxo")
                    nc.vector.tensor_copy(kxo[:, :, 0:HID], tr_ps[:, :, 0:HID])
                    nc.vector.memset(kxo[:, :, HID:HID + 1], 1.0)
                    kxos.append(kxo)

                # attention phase: scores -> exp -> weighted sum per head
                for h in range(N_HEAD):
                    qx_sb = qxs[par][h]
                    kx97_t = kx97[par][h]
                    kxo = kxos[h]
                    exp_sb = []
                    for kc in range(KC):
                        s_ps = ps_score.tile([128, SEQ], F32, tag="score", name="s_ps")
                        nc.tensor.matmul(
                            s_ps[:], kx97_t[:, kc * 128:(kc + 1) * 128],
                            qx_sb[:], start=True, stop=True)
                        e_sb = exp_pool.tile([128, SEQ], BF16, tag="exp", name="e_sb")
                        nc.scalar.activation(
                            e_sb[:], s_ps[:],
                            mybir.ActivationFunctionType.Exp, scale=SCALE)
                        exp_sb.append(e_sb)

                    # attention-weighted values + softmax denominator
                    for qc in range(QC):
                        o_ps = ps_out.tile([128, HID + 1], F32, tag="out", name="o_ps")
                        for kc in range(KC):
                            nc.tensor.matmul(
                                o_ps[:],
                                exp_sb[kc][:, qc * 128:(qc + 1) * 128],
                                kxo[:, kc, 0:HID + 1],
                                start=(kc == 0), stop=(kc == KC - 1))
                        rc = recip_pool.tile([128, 1], F32, tag="recip", name="recip")
                        nc.vector.reciprocal(rc[:], o_ps[:, HID:HID + 1])
                        nc.vector.tensor_scalar_mul(
                            st[qc][:, h * HID:(h + 1) * HID],
                            o_ps[:, 0:HID], rc[:])

                    if h == N_HEAD // 2 - 1:
                        # first half of the heads done: flush columns
                        # 0:384 so the final batch's store overlaps the
                        # remaining compute
                        for qc in range(QC):
                            nc.sync.dma_start(
                                out=out_d[b, qc * 128:(qc + 1) * 128,
                                          0:EMBED // 2],
                                in_=st[qc][:, 0:EMBED // 2])

                for qc in range(QC):
                    nc.sync.dma_start(
                        out=out_d[b, qc * 128:(qc + 1) * 128, EMBED // 2:],
                        in_=st[qc][:, EMBED // 2:])

    _split_excess_waits(nc)
    return nc


@lru_cache(maxsize=1)
def _get_nc():
    return build_bass()


def kernel(k, q, w_kx, w_qx):
    k = np.ascontiguousarray(k, dtype=np.float32)
    q = np.ascontiguousarray(q, dtype=np.float32)
    w_kx = np.ascontiguousarray(w_kx, dtype=np.float32)
    w_qx = np.ascontiguousarray(w_qx, dtype=np.float32)

    nc = _get_nc()
    in_maps = []
    for c in range(N_CORES):
        sl = slice(c * B, (c + 1) * B)
        in_maps.append({
            "k": np.ascontiguousarray(k[sl]),
            "q": np.ascontiguousarray(q[sl]),
            "w_kx": w_kx,
            "w_qx": w_qx,
        })
    res = run_bass_kernel_spmd(nc, in_maps, core_ids=list(range(N_CORES)))
    return np.concatenate([res.results[c]["out"] for c in range(N_CORES)],
                          axis=0)

